# revision 18
# baseline (speedup 1.0000x reference)
"""Trainium2 Bass kernel for nn_MultiHeadAttention_89232240541956.

Computes, for B=8, S=4096, H=1024, ATTN=1024, EXT=1152:
    x_ext = [h | broadcast(g) | l]                       [B, S, 1152]
    q = relu(x_ext @ Wq + bq); k = relu(x_ext @ Wk + bk) [B, S, 1024]
    scores = sum(q * k, -1) / 32, masked to -1e9 where mask == 1

Sharding: data-parallel over batch — core b owns batch b.

Key transformations (baseline at 218us in kernel_baseline_218.py):
  - v (Wv, bv) is dead code in the reference — skipped.
  - g @ Wq[1024:1088] folded into the bias on host; bias folded into the
    matmul as a ones-row against a bias-row.
  - 6 matmul passes per projection per tile (vs 7): fp8 E4M3 DoubleRow
    chunks cover 706 of the 1089 contraction rows (3 passes: 256+256+194),
    bf16 covers the rest (3 passes: 128+128+127 = 318 h + 64 l + bias).
    n8=706 is the minimum fp8 coverage that reaches 6 passes, minimizing
    fp8 noise. Host-simulated (sim_err.py) max rel err 1.789e-2
    (device-validated sim); gate is 2e-2. Measured on device: 1.793e-2.
  - Partial-height chunks (97/127 partitions) matmul'd directly — no
    zero padding or wasted DMA bytes (matmul cost depends on N only).
  - Epilogue uses relu(q)*relu(k) == relu(relu(q)*k): ScalarE relu on q,
    DVE multiply vs raw k PSUM, ScalarE Relu-with-accum row reduction.
  - DMA discipline (the hard-won part): only sync/scalar trigger HWDGE
    (gpsimd software DGE crawls ~24 GB/s); per-ring throughput is only
    ~25-40 GB/s so bandwidth comes from many concurrent transfers; the
    head issues ~33 transfers in exact first-use order, alternating
    engines, with the first three (c0 W-q/W-k halves, block-0 fp8 x)
    split in half again for latency. fp8 W slabs are laid out nh-major
    so half-N transfers stay contiguous. x pools are triple-buffered
    and blocks 0-2 prefetch at the head (so their triggers don't queue
    behind epilogue work); later blocks trigger on the otherwise-idle
    sync queue.
  - Scales: x*16, W*64 (powers of two: lossless in bf16, in-range for
    TRN E4M3's +-240 max); 1/(32*2^20) folded into the epilogue accum.
"""

import numpy as np
import ml_dtypes

B, S, H, LOC = 8, 4096, 1024, 64
ATTN = 1024
SBLK = 512                # seq columns per DMA block
NBLK = S // SBLK          # 8
NT = SBLK // 128          # 4 seq tiles (128 tokens) per block
NCOL = S // 128           # 32 output columns

N8 = 706                  # h-dims in fp8 (2x256 full chunks + 194-row chunk)
NC2 = 98                  # partitions in the short fp8 chunk (97 real + 1
                          # zero pad: odd-partition DMAs are ~10x slower)
NJ2 = 128                 # short bf16 chunk (62 h + 64 l + bias + 1 pad)

BF16 = ml_dtypes.bfloat16
XS = 16.0
WS = 64.0

_CACHE = {}


def _build_nc():
    import concourse.bass as bass
    import concourse.mybir as mybir
    import concourse.tile as tile
    from concourse import bacc

    dt = mybir.dt
    nc = bacc.Bacc(None, target_bir_lowering=False)

    # fp8 x: [blk*128+p, (c*2+j)*512+s] for c<2; short chunk separate.
    xh8_d = nc.dram_tensor("xh8", [NBLK * 128, 2 * 2 * SBLK], dt.float8e4,
                           kind="ExternalInput")
    xh8c2_d = nc.dram_tensor("xh8c2", [NBLK * NC2, 2 * SBLK], dt.float8e4,
                             kind="ExternalInput")
    # bf16 x: [blk*128+p, j*512+s] for j<2; short chunk separate.
    xhb_d = nc.dram_tensor("xhb", [NBLK * 128, 2 * SBLK], dt.bfloat16,
                           kind="ExternalInput")
    xhbj2_d = nc.dram_tensor("xhbj2", [NBLK * NJ2, SBLK], dt.bfloat16,
                             kind="ExternalInput")
    # W partition-major: row p holds every slab's data for partition p,
    # so each whole-W DMA is one fully contiguous [parts, cols] block.
    w8_d = nc.dram_tensor("w8", [128, 4 * 2 * 2 * 512], dt.float8e4,
                          kind="ExternalInput")
    w8c2_d = nc.dram_tensor("w8c2", [NC2, 2 * 2 * 2 * 512], dt.float8e4,
                            kind="ExternalInput")
    wb_d = nc.dram_tensor("wb", [128, 4 * ATTN], dt.bfloat16,
                          kind="ExternalInput")
    wbj2_d = nc.dram_tensor("wbj2", [NJ2, 2 * ATTN], dt.bfloat16,
                            kind="ExternalInput")
    out = nc.dram_tensor("out", [128, NCOL], dt.float32, kind="ExternalOutput")

    scale = 1.0 / (32.0 * (XS * WS) ** 2)
    DR = mybir.MatmulPerfMode.DoubleRow
    Relu = mybir.ActivationFunctionType.Relu

    with tile.TileContext(nc) as tc:
        with (
            tc.tile_pool(name="wpool", bufs=1) as wpool,
            tc.tile_pool(name="xpool", bufs=1) as xpool,
            tc.tile_pool(name="epool", bufs=2) as epool,
            tc.tile_pool(name="opool", bufs=1) as opool,
            tc.tile_pool(name="psum", bufs=1, space="PSUM") as psum,
        ):
            # [p, c, proj, nh, j, a']
            w8_sb = wpool.tile([128, 2, 2, 2, 2, 512], dt.float8e4, tag="w8")
            w8c2_sb = wpool.tile([NC2, 2, 2, 2, 512], dt.float8e4, tag="w8c2")
            wb_sb = wpool.tile([128, 2, 2, ATTN], dt.bfloat16, tag="wb")
            wbj2_sb = wpool.tile([NJ2, 2, ATTN], dt.bfloat16, tag="wbj2")

            def make_x(blk):
                xh8 = xpool.tile([128, 2, 2, SBLK], dt.float8e4, tag="xh8",
                                 bufs=2, name=f"xh8_{blk}")
                xh8c2 = xpool.tile([NC2, 2, SBLK], dt.float8e4, tag="xh8c2",
                                   bufs=2, name=f"xh8c2_{blk}")
                xhb = xpool.tile([128, 2, SBLK], dt.bfloat16, tag="xhb",
                                 bufs=2, name=f"xhb_{blk}")
                xhbj2 = xpool.tile([NJ2, SBLK], dt.bfloat16, tag="xhbj2",
                                   bufs=2, name=f"xhbj2_{blk}")
                return xh8, xh8c2, xhb, xhbj2

            def dma_x(xt, blk):
                xh8, xh8c2, xhb, xhbj2 = xt
                r0 = blk * 128
                nc.sync.dma_start(xh8[:], xh8_d[r0:r0 + 128, :])
                nc.sync.dma_start(xhb[:], xhb_d[r0:r0 + 128, :])
                nc.scalar.dma_start(
                    xh8c2[:], xh8c2_d[blk * NC2:(blk + 1) * NC2, :])
                nc.scalar.dma_start(
                    xhbj2[:], xhbj2_d[blk * NJ2:(blk + 1) * NJ2, :])

            # --- head DMAs ---
            # Wave 1: all weights (4 big partition-major contiguous
            # transfers) + block-0 x. Round-robin packet sharing means the
            # in-flight set finishes late-together, so keep this set
            # minimal. Everything on the two HWDGE engines (gpsimd's
            # software DGE crawls at ~24 GB/s). Later x blocks trigger
            # from the loop one block ahead; in-flight x stays ~2 blocks.
            x_tiles = {0: make_x(0), 1: make_x(1)}
            xh8_0, xh8c2_0, xhb_0, xhbj2_0 = x_tiles[0]
            # wave-1 in tile-0 consumption order, pairs across the two
            # engines, so the first-needed transfers get the largest
            # early bandwidth share under round-robin packet scheduling.
            nc.sync.dma_start(w8_sb[:], w8_d[:])
            nc.scalar.dma_start(xh8_0[:], xh8_d[0:128, :])
            nc.sync.dma_start(w8c2_sb[:], w8c2_d[:])
            nc.scalar.dma_start(xh8c2_0[:], xh8c2_d[0:NC2, :])
            nc.sync.dma_start(wb_sb[:], wb_d[:])
            nc.scalar.dma_start(xhb_0[:], xhb_d[0:128, :])
            nc.sync.dma_start(wbj2_sb[:], wbj2_d[:])
            nc.scalar.dma_start(xhbj2_0[:], xhbj2_d[0:NJ2, :])
            dma_x(x_tiles[1], 1)

            score_sb = opool.tile([128, NCOL], dt.float32, tag="score")
            sc2 = opool.tile([128, 2], dt.float32, tag="sc2")

            def chunk_ops(xt, s0):
                """(lhs, rhs_q, rhs_k, perf_mode) per chunk, in order."""
                xh8, xh8c2, xhb, xhbj2 = xt
                ops = []
                for c in range(2):
                    ops.append((xh8[:, c, :, s0:s0 + 128],
                                w8_sb[:, c, 0], w8_sb[:, c, 1], DR))
                ops.append((xh8c2[:, :, s0:s0 + 128],
                            w8c2_sb[:, 0], w8c2_sb[:, 1], DR))
                for j in range(2):
                    ops.append((xhb[:, j, s0:s0 + 128],
                                wb_sb[:, j, 0], wb_sb[:, j, 1], None))
                ops.append((xhbj2[:, s0:s0 + 128],
                            wbj2_sb[:, 0], wbj2_sb[:, 1], None))
                return ops

            def rsl(rhs, pm, nh):
                return rhs[:, nh] if pm else rhs[:, nh * 512:(nh + 1) * 512]

            for blk in range(NBLK):
                xt = x_tiles.pop(blk)
                nxt = blk + 2
                if nxt < NBLK:
                    x_tiles[nxt] = make_x(nxt)
                    dma_x(x_tiles[nxt], nxt)
                for t in range(NT):
                    is_last = blk == NBLK - 1 and t == NT - 1
                    col = blk * NT + t
                    psq = psum.tile([128, ATTN], dt.float32, tag="psq",
                                    bufs=2, name=f"psq_{blk}_{t}")
                    psk = psum.tile([128, ATTN], dt.float32, tag="psk",
                                    bufs=2, name=f"psk_{blk}_{t}")
                    s0 = t * 128
                    ops = chunk_ops(xt, s0)

                    if not is_last:
                        for i, (lhs, rq, rk, pm) in enumerate(ops):
                            for nh in range(2):
                                n0 = nh * 512
                                nc.tensor.matmul(
                                    psq[:, n0:n0 + 512], lhs, rsl(rq, pm, nh),
                                    start=(i == 0), stop=(i == 5),
                                    perf_mode=pm)
                                nc.tensor.matmul(
                                    psk[:, n0:n0 + 512], lhs, rsl(rk, pm, nh),
                                    start=(i == 0), stop=(i == 5),
                                    perf_mode=pm)
                        qsb = epool.tile([128, ATTN], dt.bfloat16, tag="qsb")
                        nc.scalar.activation(qsb[:], psq[:], Relu)
                        prod = epool.tile([128, ATTN], dt.bfloat16, tag="prod")
                        nc.vector.tensor_mul(prod[:], qsb[:], psk[:])
                        cpy = epool.tile([128, ATTN], dt.bfloat16, tag="cpy")
                        nc.scalar.activation(
                            cpy[:], prod[:], Relu, scale=scale,
                            accum_out=score_sb[:, col:col + 1])
                        if col == NCOL - 5:
                            # early output slab once cols 0..27 are final
                            nc.sync.dma_start(out[:, 0:28], score_sb[:, 0:28])
                    else:
                        # q-pass fully first
                        for i, (lhs, rq, rk, pm) in enumerate(ops):
                            for nh in range(2):
                                nc.tensor.matmul(
                                    psq[:, nh * 512:nh * 512 + 512], lhs,
                                    rsl(rq, pm, nh),
                                    start=(i == 0), stop=(i == 5),
                                    perf_mode=pm)
                        # k-pass; q relu overlaps the k matmuls
                        qsb = epool.tile([128, ATTN], dt.bfloat16, tag="qsb")
                        nc.scalar.activation(qsb[:], psq[:], Relu)
                        for nh in range(2):
                            for i, (lhs, rq, rk, pm) in enumerate(ops):
                                nc.tensor.matmul(
                                    psk[:, nh * 512:nh * 512 + 512], lhs,
                                    rsl(rk, pm, nh),
                                    start=(i == 0), stop=(i == 5),
                                    perf_mode=pm)
                        for nh in range(2):
                            n0 = nh * 512
                            prh = epool.tile([128, 512], dt.bfloat16,
                                             tag="prh", name=f"prh_{nh}")
                            nc.vector.tensor_mul(prh[:], qsb[:, n0:n0 + 512],
                                                 psk[:, n0:n0 + 512])
                            cph = epool.tile([128, 512], dt.bfloat16,
                                             tag="cph", name=f"cph_{nh}")
                            nc.scalar.activation(
                                cph[:], prh[:], Relu, scale=scale,
                                accum_out=sc2[:, nh:nh + 1])
                        nc.vector.tensor_reduce(
                            score_sb[:, col:col + 1], sc2[:],
                            axis=mybir.AxisListType.X, op=mybir.AluOpType.add)
                        nc.sync.dma_start(out[:, 28:32], score_sb[:, 28:32])

    nc.compile()
    return nc


def _get_nc():
    if "nc" not in _CACHE:
        _CACHE["nc"] = _build_nc()
    return _CACHE["nc"]


def prep_in_maps(h, mask, g, l, Wq, bq, Wk, bk, Wv=None, bv=None):
    import concourse.mybir as mybir

    FP8 = mybir.dt.np(mybir.dt.float8e4)

    h = np.asarray(h, dtype=np.float32)
    g = np.asarray(g, dtype=np.float32)
    l_ = np.asarray(l, dtype=np.float32)
    Wq = np.asarray(Wq, dtype=np.float32)
    bq = np.asarray(bq, dtype=np.float32)
    Wk = np.asarray(Wk, dtype=np.float32)
    bk = np.asarray(bk, dtype=np.float32)

    # Fold the per-batch g contribution into the bias (fp32 on host).
    bq_eff = bq[None, :] + g @ Wq[H:H + LOC]            # [B, ATTN]
    bk_eff = bk[None, :] + g @ Wk[H:H + LOC]

    # --- shared weights ---
    w8 = np.empty((2, 2, 128, 2, 2, 512), dtype=FP8)    # [c,proj,p,nh,j,a']
    w8c2 = np.empty((2, NC2, 2, 2, 512), dtype=FP8)     # [proj,p,nh,j,a']
    wb = np.empty((2, 2, 128, ATTN), dtype=BF16)        # [j,proj,p,a]
    wbj2_base = np.empty((2, NJ2, ATTN), dtype=np.float32)
    for proj, W in ((0, Wq), (1, Wk)):
        W8 = (W[:N8] * WS).astype(FP8)
        # rows c*256+2p+j -> [c][p][j][nh][a'] -> [c][p][nh][j][a']
        w8[:, proj] = W8[:512].reshape(2, 128, 2, 2, 512).transpose(
            0, 1, 3, 2, 4)
        w8c2[proj, :97] = W8[512:N8].reshape(97, 2, 2, 512).transpose(0, 2, 1, 3)
        w8c2[proj, 97:] = 0
        Wbf = (W[N8:H] * WS).astype(BF16)
        wb[:, proj] = Wbf[:256].reshape(2, 128, ATTN)
        wbj2_base[proj, 0:62] = W[N8 + 256:H] * WS
        wbj2_base[proj, 62:62 + LOC] = W[H + LOC:] * WS
        wbj2_base[proj, 62 + LOC + 1:] = 0
    # partition-major: [p, slab..., cols]
    base = {"w8": np.ascontiguousarray(w8.transpose(2, 0, 1, 3, 4, 5)).reshape(128, -1),
            "w8c2": np.ascontiguousarray(w8c2.transpose(1, 0, 2, 3, 4)).reshape(NC2, -1),
            "wb": np.ascontiguousarray(wb.transpose(2, 0, 1, 3)).reshape(128, -1)}

    in_maps = []
    for b in range(B):
        m = dict(base)
        hT = h[b].T                                     # [H, S]
        x8 = (hT[:N8] * XS).astype(FP8)                 # [706, S]
        # rows c*256+2p+j, cols blk*512+s -> [blk][p][c][j][s]
        m["xh8"] = np.ascontiguousarray(
            x8[:512].reshape(2, 128, 2, NBLK, SBLK).transpose(3, 1, 0, 2, 4)
        ).reshape(NBLK * 128, -1)
        xc2 = np.zeros((NC2, 2, NBLK, SBLK), dtype=FP8)
        xc2[:97] = x8[512:N8].reshape(97, 2, NBLK, SBLK)
        m["xh8c2"] = np.ascontiguousarray(
            xc2.transpose(2, 0, 1, 3)).reshape(NBLK * NC2, -1)
        xb = (hT[N8:] * XS).astype(BF16)                # [318, S]
        m["xhb"] = np.ascontiguousarray(
            xb[:256].reshape(2, 128, NBLK, SBLK).transpose(2, 1, 0, 3)
        ).reshape(NBLK * 128, -1)
        xj2 = np.zeros((NJ2, S), dtype=BF16)
        xj2[0:62] = xb[256:]
        xj2[62:62 + LOC] = l_[b].T * XS
        xj2[62 + LOC] = XS
        m["xhbj2"] = np.ascontiguousarray(
            xj2.reshape(NJ2, NBLK, SBLK).transpose(1, 0, 2)
        ).reshape(NBLK * NJ2, -1)
        wbj2 = wbj2_base.copy()
        # ones-row carries XS, so the bias row needs only WS.
        wbj2[0, 62 + LOC] = bq_eff[b] * WS
        wbj2[1, 62 + LOC] = bk_eff[b] * WS
        m["wbj2"] = np.ascontiguousarray(
            wbj2.astype(BF16).transpose(1, 0, 2)).reshape(NJ2, -1)
        in_maps.append(m)
    return in_maps


def kernel(h, mask, g, l, Wq, bq, Wk, bk, Wv=None, bv=None):
    from concourse.bass_utils import run_bass_kernel_spmd

    mask = np.asarray(mask)
    in_maps = prep_in_maps(h, mask, g, l, Wq, bq, Wk, bk)

    nc = _get_nc()
    res = run_bass_kernel_spmd(nc, in_maps, core_ids=list(range(B)), trace=False)

    scores = np.empty((B, S), dtype=np.float32)
    for b in range(B):
        scores[b] = res.results[b]["out"].T.reshape(S)
    return np.where(mask == 1, np.float32(-1e9), scores).astype(np.float32)


# revision 19
# speedup vs baseline: 1.0122x; 1.0122x over previous
"""Trainium2 Bass kernel for nn_MultiHeadAttention_89232240541956.

Computes, for B=8, S=4096, H=1024, ATTN=1024, EXT=1152:
    x_ext = [h | broadcast(g) | l]                       [B, S, 1152]
    q = relu(x_ext @ Wq + bq); k = relu(x_ext @ Wk + bk) [B, S, 1024]
    scores = sum(q * k, -1) / 32, masked to -1e9 where mask == 1

Sharding: data-parallel over batch — core b owns batch b.

Key transformations (baseline at 218us in kernel_baseline_218.py):
  - v (Wv, bv) is dead code in the reference — skipped.
  - g @ Wq[1024:1088] folded into the bias on host; bias folded into the
    matmul as a ones-row against a bias-row.
  - 6 matmul passes per projection per tile (vs 7): fp8 E4M3 DoubleRow
    chunks cover 706 of the 1089 contraction rows (3 passes: 256+256+194),
    bf16 covers the rest (3 passes: 128+128+127 = 318 h + 64 l + bias).
    n8=706 is the minimum fp8 coverage that reaches 6 passes, minimizing
    fp8 noise. Host-simulated (sim_err.py) max rel err 1.789e-2
    (device-validated sim); gate is 2e-2. Measured on device: 1.793e-2.
  - Partial-height chunks (97/127 partitions) matmul'd directly — no
    zero padding or wasted DMA bytes (matmul cost depends on N only).
  - Epilogue uses relu(q)*relu(k) == relu(relu(q)*k): ScalarE relu on q,
    DVE multiply vs raw k PSUM, ScalarE Relu-with-accum row reduction.
  - DMA discipline (the hard-won part): only sync/scalar trigger HWDGE
    (gpsimd software DGE crawls ~24 GB/s); per-ring throughput is only
    ~25-40 GB/s so bandwidth comes from many concurrent transfers; the
    head issues ~33 transfers in exact first-use order, alternating
    engines, with the first three (c0 W-q/W-k halves, block-0 fp8 x)
    split in half again for latency. fp8 W slabs are laid out nh-major
    so half-N transfers stay contiguous. x pools are triple-buffered
    and blocks 0-2 prefetch at the head (so their triggers don't queue
    behind epilogue work); later blocks trigger on the otherwise-idle
    sync queue.
  - Scales: x*16, W*64 (powers of two: lossless in bf16, in-range for
    TRN E4M3's +-240 max); 1/(32*2^20) folded into the epilogue accum.
"""

import numpy as np
import ml_dtypes

B, S, H, LOC = 8, 4096, 1024, 64
ATTN = 1024
SBLK = 512                # seq columns per DMA block
NBLK = S // SBLK          # 8
NT = SBLK // 128          # 4 seq tiles (128 tokens) per block
NCOL = S // 128           # 32 output columns

N8 = 706                  # h-dims in fp8 (2x256 full chunks + 194-row chunk)
NC2 = 98                  # partitions in the short fp8 chunk (97 real + 1
                          # zero pad: odd-partition DMAs are ~10x slower)
NJ2 = 128                 # short bf16 chunk (62 h + 64 l + bias + 1 pad)

BF16 = ml_dtypes.bfloat16
XS = 16.0
WS = 64.0

_CACHE = {}


def _build_nc():
    import concourse.bass as bass
    import concourse.mybir as mybir
    import concourse.tile as tile
    from concourse import bacc

    dt = mybir.dt
    nc = bacc.Bacc(None, target_bir_lowering=False)

    # fp8 x: [blk*128+p, (c*2+j)*512+s] for c<2; short chunk separate.
    xh8_d = nc.dram_tensor("xh8", [NBLK * 128, 2 * 2 * SBLK], dt.float8e4,
                           kind="ExternalInput")
    xh8c2_d = nc.dram_tensor("xh8c2", [NBLK * NC2, 2 * SBLK], dt.float8e4,
                             kind="ExternalInput")
    # bf16 x: [blk*128+p, j*512+s] for j<2; short chunk separate.
    xhb_d = nc.dram_tensor("xhb", [NBLK * 128, 2 * SBLK], dt.bfloat16,
                           kind="ExternalInput")
    xhbj2_d = nc.dram_tensor("xhbj2", [NBLK * NJ2, SBLK], dt.bfloat16,
                             kind="ExternalInput")
    # W partition-major: row p holds every slab's data for partition p,
    # so each whole-W DMA is one fully contiguous [parts, cols] block.
    w8_d = nc.dram_tensor("w8", [128, 4 * 2 * 2 * 512], dt.float8e4,
                          kind="ExternalInput")
    w8c2_d = nc.dram_tensor("w8c2", [NC2, 2 * 2 * 2 * 512], dt.float8e4,
                            kind="ExternalInput")
    wb_d = nc.dram_tensor("wb", [128, 4 * ATTN], dt.bfloat16,
                          kind="ExternalInput")
    wbj2_d = nc.dram_tensor("wbj2", [NJ2, 2 * ATTN], dt.bfloat16,
                            kind="ExternalInput")
    out = nc.dram_tensor("out", [128, NCOL], dt.float32, kind="ExternalOutput")

    scale = 1.0 / (32.0 * (XS * WS) ** 2)
    DR = mybir.MatmulPerfMode.DoubleRow
    Relu = mybir.ActivationFunctionType.Relu

    with tile.TileContext(nc) as tc:
        with (
            tc.tile_pool(name="wpool", bufs=1) as wpool,
            tc.tile_pool(name="xpool", bufs=1) as xpool,
            tc.tile_pool(name="epool", bufs=2) as epool,
            tc.tile_pool(name="opool", bufs=1) as opool,
            tc.tile_pool(name="psum", bufs=1, space="PSUM") as psum,
        ):
            # [p, c, proj, nh, j, a']
            w8_sb = wpool.tile([128, 2, 2, 2, 2, 512], dt.float8e4, tag="w8")
            w8c2_sb = wpool.tile([NC2, 2, 2, 2, 512], dt.float8e4, tag="w8c2")
            wb_sb = wpool.tile([128, 2, 2, ATTN], dt.bfloat16, tag="wb")
            wbj2_sb = wpool.tile([NJ2, 2, ATTN], dt.bfloat16, tag="wbj2")

            def make_x(blk):
                xh8 = xpool.tile([128, 2, 2, SBLK], dt.float8e4, tag="xh8",
                                 bufs=3, name=f"xh8_{blk}")
                xh8c2 = xpool.tile([NC2, 2, SBLK], dt.float8e4, tag="xh8c2",
                                   bufs=3, name=f"xh8c2_{blk}")
                xhb = xpool.tile([128, 2, SBLK], dt.bfloat16, tag="xhb",
                                 bufs=3, name=f"xhb_{blk}")
                xhbj2 = xpool.tile([NJ2, SBLK], dt.bfloat16, tag="xhbj2",
                                   bufs=3, name=f"xhbj2_{blk}")
                return xh8, xh8c2, xhb, xhbj2

            def dma_x(xt, blk):
                xh8, xh8c2, xhb, xhbj2 = xt
                r0 = blk * 128
                nc.sync.dma_start(xh8[:], xh8_d[r0:r0 + 128, :])
                nc.sync.dma_start(xhb[:], xhb_d[r0:r0 + 128, :])
                nc.scalar.dma_start(
                    xh8c2[:], xh8c2_d[blk * NC2:(blk + 1) * NC2, :])
                nc.scalar.dma_start(
                    xhbj2[:], xhbj2_d[blk * NJ2:(blk + 1) * NJ2, :])

            # --- head DMAs ---
            # Wave 1: all weights (4 big partition-major contiguous
            # transfers) + block-0 x. Round-robin packet sharing means the
            # in-flight set finishes late-together, so keep this set
            # minimal. Everything on the two HWDGE engines (gpsimd's
            # software DGE crawls at ~24 GB/s). Later x blocks trigger
            # from the loop one block ahead; in-flight x stays ~2 blocks.
            x_tiles = {0: make_x(0), 1: make_x(1), 2: make_x(2)}
            xh8_0, xh8c2_0, xhb_0, xhbj2_0 = x_tiles[0]
            # wave-1 in tile-0 consumption order, pairs across the two
            # engines, so the first-needed transfers get the largest
            # early bandwidth share under round-robin packet scheduling.
            nc.sync.dma_start(w8_sb[:], w8_d[:])
            nc.scalar.dma_start(xh8_0[:], xh8_d[0:128, :])
            nc.sync.dma_start(w8c2_sb[:], w8c2_d[:])
            nc.scalar.dma_start(xh8c2_0[:], xh8c2_d[0:NC2, :])
            nc.sync.dma_start(wb_sb[:], wb_d[:])
            nc.scalar.dma_start(xhb_0[:], xhb_d[0:128, :])
            nc.sync.dma_start(wbj2_sb[:], wbj2_d[:])
            nc.scalar.dma_start(xhbj2_0[:], xhbj2_d[0:NJ2, :])
            dma_x(x_tiles[1], 1)
            dma_x(x_tiles[2], 2)

            score_sb = opool.tile([128, NCOL], dt.float32, tag="score")
            sc2 = opool.tile([128, 2], dt.float32, tag="sc2")

            def chunk_ops(xt, s0):
                """(lhs, rhs_q, rhs_k, perf_mode) per chunk, in order."""
                xh8, xh8c2, xhb, xhbj2 = xt
                ops = []
                for c in range(2):
                    ops.append((xh8[:, c, :, s0:s0 + 128],
                                w8_sb[:, c, 0], w8_sb[:, c, 1], DR))
                ops.append((xh8c2[:, :, s0:s0 + 128],
                            w8c2_sb[:, 0], w8c2_sb[:, 1], DR))
                for j in range(2):
                    ops.append((xhb[:, j, s0:s0 + 128],
                                wb_sb[:, j, 0], wb_sb[:, j, 1], None))
                ops.append((xhbj2[:, s0:s0 + 128],
                            wbj2_sb[:, 0], wbj2_sb[:, 1], None))
                return ops

            def rsl(rhs, pm, nh):
                return rhs[:, nh] if pm else rhs[:, nh * 512:(nh + 1) * 512]

            for blk in range(NBLK):
                xt = x_tiles.pop(blk)
                nxt = blk + 3
                if nxt < NBLK:
                    x_tiles[nxt] = make_x(nxt)
                    dma_x(x_tiles[nxt], nxt)
                for t in range(NT):
                    is_last = blk == NBLK - 1 and t == NT - 1
                    col = blk * NT + t
                    psq = [psum.tile([128, 512], dt.float32, tag=f"psq{h}",
                                     bufs=2, name=f"psq{h}_{blk}_{t}")
                           for h in range(2)]
                    psk = [psum.tile([128, 512], dt.float32, tag=f"psk{h}",
                                     bufs=2, name=f"psk{h}_{blk}_{t}")
                           for h in range(2)]
                    s0 = t * 128
                    ops = chunk_ops(xt, s0)

                    qsb = epool.tile([128, ATTN], dt.bfloat16, tag="qsb")
                    prod = epool.tile([128, ATTN], dt.bfloat16, tag="prod")
                    cpy = epool.tile([128, ATTN], dt.bfloat16, tag="cpy")
                    sc2t = epool.tile([128, 2], dt.float32, tag="sc2")

                    def epi_half(nh):
                        n0 = nh * 512
                        nc.scalar.activation(qsb[:, n0:n0 + 512], psq[nh][:],
                                             Relu)
                        nc.vector.tensor_mul(prod[:, n0:n0 + 512],
                                             qsb[:, n0:n0 + 512], psk[nh][:])
                        nc.scalar.activation(
                            cpy[:, n0:n0 + 512], prod[:, n0:n0 + 512], Relu,
                            scale=scale, accum_out=sc2t[:, nh:nh + 1])

                    if not is_last:
                        for i, (lhs, rq, rk, pm) in enumerate(ops):
                            for nh in range(2):
                                nc.tensor.matmul(
                                    psq[nh][:], lhs, rsl(rq, pm, nh),
                                    start=(i == 0), stop=(i == 5),
                                    perf_mode=pm)
                                nc.tensor.matmul(
                                    psk[nh][:], lhs, rsl(rk, pm, nh),
                                    start=(i == 0), stop=(i == 5),
                                    perf_mode=pm)
                        for nh in range(2):
                            epi_half(nh)
                        nc.vector.tensor_reduce(
                            score_sb[:, col:col + 1], sc2t[:],
                            axis=mybir.AxisListType.X, op=mybir.AluOpType.add)
                        if col == NCOL - 5:
                            # early output slab once cols 0..27 are final
                            nc.sync.dma_start(out[:, 0:28], score_sb[:, 0:28])
                    else:
                        # q-pass fully first; then k per half so the h0
                        # epilogue overlaps the k-h1 matmuls.
                        for i, (lhs, rq, rk, pm) in enumerate(ops):
                            for nh in range(2):
                                nc.tensor.matmul(
                                    psq[nh][:], lhs, rsl(rq, pm, nh),
                                    start=(i == 0), stop=(i == 5),
                                    perf_mode=pm)
                        for nh in range(2):
                            for i, (lhs, rq, rk, pm) in enumerate(ops):
                                nc.tensor.matmul(
                                    psk[nh][:], lhs, rsl(rk, pm, nh),
                                    start=(i == 0), stop=(i == 5),
                                    perf_mode=pm)
                            epi_half(nh)
                        nc.vector.tensor_reduce(
                            score_sb[:, col:col + 1], sc2t[:],
                            axis=mybir.AxisListType.X, op=mybir.AluOpType.add)
                        nc.sync.dma_start(out[:, 28:32], score_sb[:, 28:32])

    nc.compile()
    return nc


def _get_nc():
    if "nc" not in _CACHE:
        _CACHE["nc"] = _build_nc()
    return _CACHE["nc"]


def prep_in_maps(h, mask, g, l, Wq, bq, Wk, bk, Wv=None, bv=None):
    import concourse.mybir as mybir

    FP8 = mybir.dt.np(mybir.dt.float8e4)

    h = np.asarray(h, dtype=np.float32)
    g = np.asarray(g, dtype=np.float32)
    l_ = np.asarray(l, dtype=np.float32)
    Wq = np.asarray(Wq, dtype=np.float32)
    bq = np.asarray(bq, dtype=np.float32)
    Wk = np.asarray(Wk, dtype=np.float32)
    bk = np.asarray(bk, dtype=np.float32)

    # Fold the per-batch g contribution into the bias (fp32 on host).
    bq_eff = bq[None, :] + g @ Wq[H:H + LOC]            # [B, ATTN]
    bk_eff = bk[None, :] + g @ Wk[H:H + LOC]

    # --- shared weights ---
    w8 = np.empty((2, 2, 128, 2, 2, 512), dtype=FP8)    # [c,proj,p,nh,j,a']
    w8c2 = np.empty((2, NC2, 2, 2, 512), dtype=FP8)     # [proj,p,nh,j,a']
    wb = np.empty((2, 2, 128, ATTN), dtype=BF16)        # [j,proj,p,a]
    wbj2_base = np.empty((2, NJ2, ATTN), dtype=np.float32)
    for proj, W in ((0, Wq), (1, Wk)):
        W8 = (W[:N8] * WS).astype(FP8)
        # rows c*256+2p+j -> [c][p][j][nh][a'] -> [c][p][nh][j][a']
        w8[:, proj] = W8[:512].reshape(2, 128, 2, 2, 512).transpose(
            0, 1, 3, 2, 4)
        w8c2[proj, :97] = W8[512:N8].reshape(97, 2, 2, 512).transpose(0, 2, 1, 3)
        w8c2[proj, 97:] = 0
        Wbf = (W[N8:H] * WS).astype(BF16)
        wb[:, proj] = Wbf[:256].reshape(2, 128, ATTN)
        wbj2_base[proj, 0:62] = W[N8 + 256:H] * WS
        wbj2_base[proj, 62:62 + LOC] = W[H + LOC:] * WS
        wbj2_base[proj, 62 + LOC + 1:] = 0
    # partition-major: [p, slab..., cols]
    base = {"w8": np.ascontiguousarray(w8.transpose(2, 0, 1, 3, 4, 5)).reshape(128, -1),
            "w8c2": np.ascontiguousarray(w8c2.transpose(1, 0, 2, 3, 4)).reshape(NC2, -1),
            "wb": np.ascontiguousarray(wb.transpose(2, 0, 1, 3)).reshape(128, -1)}

    in_maps = []
    for b in range(B):
        m = dict(base)
        hT = h[b].T                                     # [H, S]
        x8 = (hT[:N8] * XS).astype(FP8)                 # [706, S]
        # rows c*256+2p+j, cols blk*512+s -> [blk][p][c][j][s]
        m["xh8"] = np.ascontiguousarray(
            x8[:512].reshape(2, 128, 2, NBLK, SBLK).transpose(3, 1, 0, 2, 4)
        ).reshape(NBLK * 128, -1)
        xc2 = np.zeros((NC2, 2, NBLK, SBLK), dtype=FP8)
        xc2[:97] = x8[512:N8].reshape(97, 2, NBLK, SBLK)
        m["xh8c2"] = np.ascontiguousarray(
            xc2.transpose(2, 0, 1, 3)).reshape(NBLK * NC2, -1)
        xb = (hT[N8:] * XS).astype(BF16)                # [318, S]
        m["xhb"] = np.ascontiguousarray(
            xb[:256].reshape(2, 128, NBLK, SBLK).transpose(2, 1, 0, 3)
        ).reshape(NBLK * 128, -1)
        xj2 = np.zeros((NJ2, S), dtype=BF16)
        xj2[0:62] = xb[256:]
        xj2[62:62 + LOC] = l_[b].T * XS
        xj2[62 + LOC] = XS
        m["xhbj2"] = np.ascontiguousarray(
            xj2.reshape(NJ2, NBLK, SBLK).transpose(1, 0, 2)
        ).reshape(NBLK * NJ2, -1)
        wbj2 = wbj2_base.copy()
        # ones-row carries XS, so the bias row needs only WS.
        wbj2[0, 62 + LOC] = bq_eff[b] * WS
        wbj2[1, 62 + LOC] = bk_eff[b] * WS
        m["wbj2"] = np.ascontiguousarray(
            wbj2.astype(BF16).transpose(1, 0, 2)).reshape(NJ2, -1)
        in_maps.append(m)
    return in_maps


def kernel(h, mask, g, l, Wq, bq, Wk, bk, Wv=None, bv=None):
    from concourse.bass_utils import run_bass_kernel_spmd

    mask = np.asarray(mask)
    in_maps = prep_in_maps(h, mask, g, l, Wq, bq, Wk, bk)

    nc = _get_nc()
    res = run_bass_kernel_spmd(nc, in_maps, core_ids=list(range(B)), trace=False)

    scores = np.empty((B, S), dtype=np.float32)
    for b in range(B):
        scores[b] = res.results[b]["out"].T.reshape(S)
    return np.where(mask == 1, np.float32(-1e9), scores).astype(np.float32)
